# revision 18
# baseline (speedup 1.0000x reference)
"""Causal self-attention with RoPE on 8 Trainium2 NeuronCores.

Problem shapes (hardcoded): B=4, T=2048, C=1024, H=16 heads, D=64.
Sharding: data-parallel on B (4) x tensor-parallel on heads (2 groups of 8)
-> 8 cores. Each core computes, for its batch b and its 8 heads:
  qkv^ = x_b @ Wqkv[:, cols(heads)] + b_qkv[cols]
  rope(q), rope(k); causal softmax(q k^T / 8) @ v
  partial = y_heads @ Wproj[rows(heads), :]
Host gathers: out[b] = partial[b,g0] + partial[b,g1] + b_proj.

All matmuls run as float32r (full-rate fp32 mode on the PE).
"""

import math

import numpy as np

import concourse.bass as bass
import concourse.mybir as mybir
import concourse.tile as tile
from concourse import bacc
from concourse.masks import make_identity

B, T, C = 4, 2048, 1024
H = 16          # total heads
D = C // H      # 64 head dim
HL = 8          # heads per core (local)
CL = HL * D     # 512 local head-dim columns
N_CORES = 8
P = 128
TT = T // P     # 16 t tiles
QC = T // 512   # 4 q chunks of 512
F32 = mybir.dt.float32
F32R = mybir.dt.float32r

_CACHE = {}


def build_nc(use_bias=False, repeat=1):
    """Build + compile the SPMD single-core program (same on all 8 cores).

    repeat>1 unrolls the whole computation R times in one launch — used
    only for timing (per-run exec = (t_R - t_1)/(R-1), dispatch cancels).
    """
    nc = bacc.Bacc("TRN2", target_bir_lowering=False, debug=False)

    xt_d = nc.dram_tensor("xt_in", [C, T], F32R, kind="ExternalInput").ap()
    wqkv_d = nc.dram_tensor("wqkv", [C, 3 * CL], F32R, kind="ExternalInput").ap()
    bqkv_d = nc.dram_tensor("bqkv", [3 * CL], F32R, kind="ExternalInput").ap()
    wproj_d = nc.dram_tensor("wproj", [CL, C], F32R, kind="ExternalInput").ap()
    cos_d = nc.dram_tensor("cos_t", [T, D // 2], F32, kind="ExternalInput").ap()
    sin_d = nc.dram_tensor("sin_t", [T, D // 2], F32, kind="ExternalInput").ap()
    mask_d = nc.dram_tensor("masks", [P, 1920], F32R, kind="ExternalInput").ap()
    out_d = nc.dram_tensor("out_p", [T, C], F32, kind="ExternalOutput").ap()

    with tile.TileContext(nc) as tc:
        for _rep in range(repeat):
            _build_body(nc, tc, use_bias, _rep, xt_d, wqkv_d, bqkv_d,
                        wproj_d, cos_d, sin_d, mask_d, out_d)

    nc.compile()
    return nc


def _build_body(nc, tc, use_bias, _rep, xt_d, wqkv_d, bqkv_d, wproj_d,
                cos_d, sin_d, mask_d, out_d):
    if True:
        with tc.tile_pool(name=f"persist{_rep}", bufs=1) as persist:
            # persistent SBUF tensors
            ident = persist.tile([P, P], F32)
            make_identity(nc, ident)
            ident_r = persist.tile([P, P], F32R)
            nc.vector.tensor_copy(ident_r[:], ident[:])
            cos_sb = persist.tile([P, TT, D // 2], F32)   # [t%128, t//128, i]
            sin_sb = persist.tile([P, TT, D // 2], F32)
            nc.gpsimd.dma_start(cos_sb[:], cos_d.rearrange("(n p) i -> p n i", p=P))
            nc.gpsimd.dma_start(sin_sb[:], sin_d.rearrange("(n p) i -> p n i", p=P))
            onezero = persist.tile([P, 2], F32)
            nc.vector.memset(onezero[:, 0:1], 1.0)
            nc.vector.memset(onezero[:, 1:2], 0.0)

            v_sb = persist.tile([P, TT, HL, D + 2], F32R)    # [k%128, k//128, h, d|1]
            qT = persist.tile([P, HL // 2, T], F32R)         # [dim in pair, pair, t]
            kT = persist.tile([P, HL // 2, T], F32R)

            # ---------------- phase 1: qkv projection + rope + transposes ----
            with (
                tc.tile_pool(name="wpool", bufs=1) as wpool,
                tc.tile_pool(name="xt", bufs=5) as xtpool,
                tc.tile_pool(name="qkvp", bufs=5, space="PSUM") as qkvp,
                tc.tile_pool(name="rope_tmp", bufs=3) as rtmp,
                tc.tile_pool(name="rot", bufs=3) as rotpool,
                tc.tile_pool(name="trp", bufs=3, space="PSUM") as trp,
            ):
                xt_r = xt_d.rearrange("(ko p) t -> p ko t", p=P)
                xt_tiles = {}

                def load_xt(tt):
                    xt_t = xtpool.tile(
                        [P, C // P, P], F32R, name=f"xt_{tt}", tag="xt"
                    )
                    nc.sync.dma_start(xt_t[:], xt_r[:, :, tt * P : (tt + 1) * P])
                    xt_tiles[tt] = xt_t

                GA = 4  # startup group: ko-outer over first GA tiles
                for tt in range(GA):
                    load_xt(tt)
                w_sb = wpool.tile([P, C // P, 3 * CL], F32R)  # 48KB/part
                wqkv_r = wqkv_d.rearrange("(ko p) n -> p ko n", p=P)
                # chunk-major arrival order: the startup group's Q matmuls
                # only need chunk 0, so land it before K/V weight columns
                for ch in range(3):
                    for ko in range(C // P):
                        nc.scalar.dma_start(
                            w_sb[:, ko, ch * 512 : (ch + 1) * 512],
                            wqkv_r[:, ko, ch * 512 : (ch + 1) * 512],
                        )
                if use_bias:
                    b_bc = wpool.tile([P, 3 * CL], F32R)
                    nc.sync.dma_start(
                        b_bc[:], bqkv_d[None, :].to_broadcast((P, 3 * CL))
                    )

                def postproc(tt, ch, ps):
                    if use_bias:
                        nc.vector.tensor_add(
                            ps[:], ps[:], b_bc[:, ch * 512 : (ch + 1) * 512]
                        )
                    if ch < 2:  # Q or K: rope on DVE straight from PSUM
                        eng = nc.vector
                        pv = ps.rearrange(
                            "p (h i two) -> p h i two", h=HL, two=2
                        )
                        e_, o_ = pv[:, :, :, 0], pv[:, :, :, 1]
                        cosb = cos_sb[:, tt, None, :].to_broadcast((P, HL, D // 2))
                        sinb = sin_sb[:, tt, None, :].to_broadcast((P, HL, D // 2))
                        rot = rotpool.tile([P, HL, D], F32R)
                        re_, ro_ = rot[:, :, 0 : D // 2], rot[:, :, D // 2 : D]
                        t1 = rtmp.tile([P, HL, D // 2], F32, tag="t1")
                        t2 = rtmp.tile([P, HL, D // 2], F32, tag="t2")
                        t3 = rtmp.tile([P, HL, D // 2], F32, tag="t3")
                        t4 = rtmp.tile([P, HL, D // 2], F32, tag="t4")
                        eng.tensor_mul(t1[:], e_, cosb)
                        eng.tensor_mul(t2[:], o_, sinb)
                        eng.tensor_mul(t3[:], e_, sinb)
                        eng.tensor_mul(t4[:], o_, cosb)
                        eng.tensor_sub(re_, t1[:], t2[:])
                        eng.tensor_add(ro_, t3[:], t4[:])
                        dstT = qT if ch == 0 else kT
                        rflat = rot.rearrange("p h d -> p (h d)")
                        tp2 = trp.tile([P, 512], F32R)
                        for pr in range(HL // 2):
                            nc.tensor.matmul(
                                tp2[:, pr * P : (pr + 1) * P],
                                rflat[:, pr * P : (pr + 1) * P],
                                ident_r[:],
                                is_transpose=True,
                                start=(pr == 0),
                                stop=(pr == 3),
                            )
                        nc.scalar.copy(
                            dstT[:, :, tt * P : (tt + 1) * P],
                            tp2.rearrange("p (b q) -> p b q", b=4),
                        )
                    else:  # V: copy into [t, h, d] layout + ones column
                        nc.scalar.copy(
                            v_sb[:, tt, :, 0:D],
                            ps.rearrange("p (h d) -> p h d", h=HL),
                        )
                        nc.vector.tensor_copy(
                            v_sb[:, tt, :, D : D + 2],
                            onezero[:, None, :].to_broadcast((P, HL, 2)),
                        )

                # startup group: ko-outer so each arriving 256KB weight block
                # feeds GA matmuls — PE overlaps the initial weight DMA
                # stream instead of stalling on the full 6MB load.
                for ch in range(3):
                    ga_ps = [
                        qkvp.tile([P, 512], F32, name=f"ga{ch}_{t}", tag="qkv")
                        for t in range(GA)
                    ]
                    for kb in range(C // P):
                        for t in range(GA):
                            nc.tensor.matmul(
                                ga_ps[t][:],
                                xt_tiles[t][:, kb, :],
                                w_sb[:, kb, ch * 512 : (ch + 1) * 512],
                                start=(kb == 0),
                                stop=(kb == C // P - 1),
                            )
                    for t in range(GA):
                        postproc(t, ch, ga_ps[t])
                for t in range(GA):
                    xt_tiles.pop(t)
                for tt in range(GA, min(GA + 3, TT)):
                    load_xt(tt)
                for tt in range(GA, TT):
                    if tt + 3 < TT:
                        load_xt(tt + 3)
                    xt_t = xt_tiles.pop(tt)
                    for ch in range(3):
                        ps = qkvp.tile([P, 512], F32, tag="qkv")
                        for kb in range(C // P):
                            nc.tensor.matmul(
                                ps[:],
                                xt_t[:, kb, :],
                                w_sb[:, kb, ch * 512 : (ch + 1) * 512],
                                start=(kb == 0),
                                stop=(kb == C // P - 1),
                            )
                        postproc(tt, ch, ps)

            # ------- phase 2+3: attention (qc-outer) + interleaved projection --
            with tc.tile_pool(name="p2", bufs=1) as p2:
                yT = p2.tile([P, HL // 2, T], F32R)
                # additive master causal mask: mm[p, c] = 0 iff c >= p + 896
                # else -1e30. slice [:, 896-r : 896-r+W] masks [j < p + r].
                mm = p2.tile([P, 1920], F32R)
                nc.gpsimd.dma_start(mm[:], mask_d[:])
                wp_sb = p2.tile([P, CL // P, C], F32R)
                for s in range(CL // P):
                    nc.sync.dma_start(
                        wp_sb[:, s, :],
                        wproj_d.rearrange("(s p) n -> p s n", p=P)[:, s, :],
                    )
                with (
                    tc.tile_pool(name="sp", bufs=2, space="PSUM") as spool,
                    tc.tile_pool(name="yp", bufs=1, space="PSUM") as ypool,
                    tc.tile_pool(name="pp", bufs=2, space="PSUM") as ppool,
                    tc.tile_pool(name="ep", bufs=5) as epool,
                    tc.tile_pool(name="rp", bufs=3) as rpool,
                    tc.tile_pool(name="op", bufs=2) as opool,
                ):
                    for qc in range(2):  # q chunks of 1024
                        for h in range(HL):
                            pr, po = h // 2, (h % 2) * D
                            q0 = qc * 1024
                            nk = 8 * (qc + 1)      # k tiles for half1
                            nk0 = nk - 4           # k tiles for half0
                            qTh = [
                                qT[po : po + D, pr, q0 + i * 512 : q0 + (i + 1) * 512]
                                for i in range(2)
                            ]
                            y_ps = [
                                ypool.tile(
                                    [D + 2, 512], F32, tag=f"y{i}", name=f"y_ps{i}"
                                )
                                for i in range(2)
                            ]
                            for kt in range(nk):
                                kslice = kT[po : po + D, pr, kt * P : (kt + 1) * P]
                                s_ps = spool.tile([P, 1024], F32)
                                r = kt * P - q0    # mask offset vs half0
                                exp_lo = None
                                if kt < nk0:
                                    # leading s0 cols of half0 are fully
                                    # masked (p < 128 <= r... j < p + r for
                                    # all j < r): skip them in scores, mask,
                                    # exp and PV. Keep remaining width >= 256
                                    # for full-rate fp32r.
                                    s0 = min(max(r, 0), 256)
                                    nc.tensor.matmul(
                                        s_ps[:, s0:512], kslice,
                                        qTh[0][:, s0:512],
                                        start=True, stop=(r < 0),
                                    )
                                    if r >= 0:
                                        # additive mask as a second matmul in
                                        # the same accumulation group
                                        re = r - s0  # residual offset (0/128)
                                        nc.tensor.matmul(
                                            s_ps[:, s0:512],
                                            ident_r[:],
                                            mm[:, 896 - re : 1408 - re - s0],
                                            start=False, stop=True,
                                        )
                                    # half1 of full tiles never needs masking
                                    # (r <= 384, so p + r < 512 <= j)
                                    nc.tensor.matmul(
                                        s_ps[:, 512:1024], kslice, qTh[1],
                                        start=True, stop=True,
                                    )
                                    exp_lo = s0
                                else:
                                    r1 = r - 512   # offset vs half1 (0..384)
                                    s1 = min(r1, 256)
                                    re = r1 - s1
                                    nc.tensor.matmul(
                                        s_ps[:, 512 + s1 : 1024], kslice,
                                        qTh[1][:, s1:512],
                                        start=True, stop=False,
                                    )
                                    nc.tensor.matmul(
                                        s_ps[:, 512 + s1 : 1024],
                                        ident_r[:],
                                        mm[:, 896 - re : 1408 - re - s1],
                                        start=False, stop=True,
                                    )
                                    exp_lo = 512 + s1
                                e_sb = epool.tile([P, 1024], F32R)
                                nc.scalar.activation(
                                    e_sb[:, exp_lo:1024], s_ps[:, exp_lo:1024],
                                    mybir.ActivationFunctionType.Exp,
                                    scale=1.0 / math.sqrt(D),
                                )
                                vsl = v_sb[:, kt, h, :]
                                if kt < nk0:
                                    nc.tensor.matmul(
                                        y_ps[0][:, s0:512], vsl,
                                        e_sb[:, s0:512],
                                        start=(kt == 0), stop=(kt == nk0 - 1),
                                    )
                                    pv1_lo = 0
                                else:
                                    pv1_lo = s1
                                nc.tensor.matmul(
                                    y_ps[1][:, pv1_lo:512], vsl,
                                    e_sb[:, 512 + pv1_lo : 1024],
                                    start=(kt == 0), stop=(kt == nk - 1),
                                )
                            for i in range(2):
                                recip = rpool.tile([1, 512], F32, tag="recip")
                                nc.vector.reciprocal(recip[:], y_ps[i][D : D + 1, :])
                                rb = rpool.tile([D, 512], F32, tag="rb")
                                nc.gpsimd.partition_broadcast(rb[:], recip[:])
                                qh0 = q0 + i * 512
                                nc.vector.tensor_mul(
                                    yT[po : po + D, pr, qh0 : qh0 + 512],
                                    y_ps[i][0:D, :],
                                    rb[:],
                                )
                        # projection for the t range this q chunk covers
                        for tt in range(qc * 8, qc * 8 + 8):
                            for nch in range(2):
                                pps = ppool.tile([P, 512], F32)
                                for s in range(CL // P):
                                    nc.tensor.matmul(
                                        pps[:],
                                        yT[:, s, tt * P : (tt + 1) * P],
                                        wp_sb[:, s, nch * 512 : (nch + 1) * 512],
                                        start=(s == 0),
                                        stop=(s == CL // P - 1),
                                    )
                                o_sb = opool.tile([P, 512], F32)
                                nc.vector.tensor_copy(o_sb[:], pps[:])
                                nc.sync.dma_start(
                                    out_d[
                                        tt * P : (tt + 1) * P,
                                        nch * 512 : (nch + 1) * 512,
                                    ],
                                    o_sb[:],
                                )


def _round_f32r(a):
    """Round fp32 to the fp32r storage format: RNE to 11 mantissa bits
    (low 12 mantissa bits zero)."""
    u = np.ascontiguousarray(a, dtype=np.float32).view(np.uint32)
    r = u + (0x7FF + ((u >> 12) & 1))
    r &= np.uint32(0xFFFFF000)
    return r.view(np.float32)


def _host_consts():
    i = np.arange(D // 2, dtype=np.float32)
    theta = (10000.0 ** (-2.0 * i / D)).astype(np.float32)
    pos = np.arange(T, dtype=np.float32)[:, None]
    ang = pos * theta[None, :]
    cos_t = np.cos(ang).astype(np.float32)
    sin_t = np.sin(ang).astype(np.float32)
    p = np.arange(P)[:, None]
    c = np.arange(1920)[None, :]
    # additive master causal mask: 0 where valid, -1e30 where masked
    masks = np.where(c >= p + 896, 0.0, -1e30).astype(np.float32)
    return cos_t, sin_t, masks


def _make_in_maps(x, W_qkv, b_qkv, W_proj):
    cos_t, sin_t, masks = _host_consts()
    in_maps = []
    for core in range(N_CORES):
        b = core // 2
        g = core % 2
        cols = np.arange(g * CL, (g + 1) * CL)  # local head cols within Q/K/V
        wq = W_qkv[:, cols]
        wk = W_qkv[:, C + cols]
        wv = W_qkv[:, 2 * C + cols]
        wqkv = _round_f32r(np.concatenate([wq, wk, wv], axis=1))
        bq = _round_f32r(
            np.concatenate([b_qkv[cols], b_qkv[C + cols], b_qkv[2 * C + cols]])
        )
        wproj = _round_f32r(W_proj[cols, :])
        in_maps.append(
            {
                "xt_in": _round_f32r(x[b].T),
                "wqkv": wqkv,
                "bqkv": np.ascontiguousarray(bq),
                "wproj": wproj,
                "cos_t": cos_t,
                "sin_t": sin_t,
                "masks": masks,
            }
        )
    return in_maps


class _Runner:
    """Cached jitted shard_map executable over the 8 NeuronCores."""

    def __init__(self, nc, n_cores):
        import jax
        from jax.sharding import Mesh, PartitionSpec, NamedSharding
        from jax.experimental.shard_map import shard_map
        from concourse import bass2jax

        bass2jax.install_neuronx_cc_hook()
        self.n_cores = n_cores
        partition_name = (
            nc.partition_id_tensor.name if nc.partition_id_tensor else None
        )
        in_names, out_names, out_avals, zeros = [], [], [], []
        for alloc in nc.m.functions[0].allocations:
            if not isinstance(alloc, mybir.MemoryLocationSet):
                continue
            name = alloc.memorylocations[0].name
            if alloc.kind == "ExternalInput":
                if name != partition_name:
                    in_names.append(name)
            elif alloc.kind == "ExternalOutput":
                shape = tuple(alloc.tensor_shape)
                dtype = mybir.dt.np(alloc.dtype)
                out_names.append(name)
                out_avals.append(jax.core.ShapedArray(shape, dtype))
                zeros.append(np.zeros(shape, dtype))
        self.in_names, self.out_names, self.out_avals = (
            in_names,
            out_names,
            out_avals,
        )
        all_in = list(in_names) + list(out_names)
        if partition_name is not None:
            all_in.append(partition_name)

        def _body(*args):
            operands = list(args)
            if partition_name is not None:
                operands.append(bass2jax.partition_id_tensor())
            return tuple(
                bass2jax._bass_exec_p.bind(
                    *operands,
                    out_avals=tuple(out_avals),
                    in_names=tuple(all_in),
                    out_names=tuple(out_names),
                    lowering_input_output_aliases=(),
                    sim_require_finite=True,
                    sim_require_nnan=True,
                    nc=nc,
                )
            )

        devices = jax.devices()[:n_cores]
        mesh = Mesh(np.asarray(devices), ("core",))
        spec = PartitionSpec("core")
        self.sharding = NamedSharding(mesh, spec)
        n_outs = len(out_names)
        self.fn = jax.jit(
            shard_map(
                _body,
                mesh=mesh,
                in_specs=(spec,) * (len(in_names) + n_outs),
                out_specs=(spec,) * n_outs,
                check_rep=False,
            ),
            keep_unused=True,
        )
        self._jax = jax
        self.zeros_dev = [
            jax.device_put(
                np.zeros((n_cores * z.shape[0], *z.shape[1:]), z.dtype),
                self.sharding,
            )
            for z in zeros
        ]

    def run(self, in_maps):
        jax = self._jax
        concat = [
            np.concatenate([np.asarray(m[name]) for m in in_maps], axis=0)
            for name in self.in_names
        ]
        dev_in = [jax.device_put(a, self.sharding) for a in concat]
        outs = self.fn(*dev_in, *self.zeros_dev)
        results = []
        for c in range(self.n_cores):
            d = {}
            for i, name in enumerate(self.out_names):
                shape = self.out_avals[i].shape
                d[name] = np.asarray(outs[i]).reshape(self.n_cores, *shape)[c]
            results.append(d)
        return results


def kernel(x, W_qkv, b_qkv, W_proj, b_proj):
    x = np.asarray(x, dtype=np.float32)
    W_qkv = np.asarray(W_qkv, dtype=np.float32)
    b_qkv = np.asarray(b_qkv, dtype=np.float32)
    W_proj = np.asarray(W_proj, dtype=np.float32)
    b_proj = np.asarray(b_proj, dtype=np.float32)

    use_bias = bool(np.any(b_qkv))
    key = ("runner", use_bias)
    if key not in _CACHE:
        _CACHE[("nc", use_bias)] = build_nc(use_bias)
        _CACHE[key] = _Runner(_CACHE[("nc", use_bias)], N_CORES)
    runner = _CACHE[key]

    in_maps = _make_in_maps(x, W_qkv, b_qkv, W_proj)
    results = runner.run(in_maps)
    out = np.empty((B, T, C), dtype=np.float32)
    for b in range(B):
        out[b] = results[2 * b]["out_p"] + results[2 * b + 1]["out_p"]
    out += b_proj[None, None, :]
    return out



# revision 26
# speedup vs baseline: 1.0892x; 1.0892x over previous
"""Causal self-attention with RoPE on 8 Trainium2 NeuronCores.

Problem shapes (hardcoded): B=4, T=2048, C=1024, H=16 heads, D=64.
Sharding: data-parallel on B (4) x tensor-parallel on heads (2 groups of 8)
-> 8 cores. Each core computes, for its batch b and its 8 heads:
  qkv^ = x_b @ Wqkv[:, cols(heads)] + b_qkv[cols]
  rope(q), rope(k); causal softmax(q k^T / 8) @ v
  partial = y_heads @ Wproj[rows(heads), :]
Host gathers: out[b] = partial[b,g0] + partial[b,g1] + b_proj.

All matmuls run as float32r (full-rate fp32 mode on the PE).
"""

import math

import numpy as np

import concourse.bass as bass
import concourse.mybir as mybir
import concourse.tile as tile
from concourse import bacc
from concourse.masks import make_identity

B, T, C = 4, 2048, 1024
H = 16          # total heads
D = C // H      # 64 head dim
HL = 8          # heads per core (local)
CL = HL * D     # 512 local head-dim columns
N_CORES = 8
P = 128
TT = T // P     # 16 t tiles
QC = T // 512   # 4 q chunks of 512
F32 = mybir.dt.float32
F32R = mybir.dt.float32r

_CACHE = {}
EXPERIMENT = 0  # 0=normal, 1=no-exp (pv from const), 2=exp-no-pv-dep


def build_nc(use_bias=False, repeat=1, phases=3):
    """Build + compile the SPMD single-core program (same on all 8 cores).

    repeat>1 unrolls the whole computation R times in one launch — used
    only for timing (per-run exec = (t_R - t_1)/(R-1), dispatch cancels).
    phases: 1 = qkv+rope only, 2 = +attention, 3 = +projection (full).
    Partial builds produce garbage outputs; timing-only.
    """
    nc = bacc.Bacc("TRN2", target_bir_lowering=False, debug=False)

    xt_d = nc.dram_tensor("xt_in", [C, T], F32R, kind="ExternalInput").ap()
    wqkv_d = nc.dram_tensor("wqkv", [C, 3 * CL], F32R, kind="ExternalInput").ap()
    bqkv_d = nc.dram_tensor("bqkv", [3 * CL], F32R, kind="ExternalInput").ap()
    wproj_d = nc.dram_tensor("wproj", [CL, C], F32R, kind="ExternalInput").ap()
    cos_d = nc.dram_tensor("cos_t", [T, D // 2], F32, kind="ExternalInput").ap()
    sin_d = nc.dram_tensor("sin_t", [T, D // 2], F32, kind="ExternalInput").ap()
    mask_d = nc.dram_tensor("masks", [P, 1920], F32R, kind="ExternalInput").ap()
    out_d = nc.dram_tensor("out_p", [T, C], F32, kind="ExternalOutput").ap()

    with tile.TileContext(nc) as tc:
        for _rep in range(repeat):
            _build_body(nc, tc, use_bias, _rep, xt_d, wqkv_d, bqkv_d,
                        wproj_d, cos_d, sin_d, mask_d, out_d, phases)

    nc.compile()
    return nc


def _build_body(nc, tc, use_bias, _rep, xt_d, wqkv_d, bqkv_d, wproj_d,
                cos_d, sin_d, mask_d, out_d, phases=3):
    if True:
        with tc.tile_pool(name=f"persist{_rep}", bufs=1) as persist:
            # persistent SBUF tensors
            ident = persist.tile([P, P], F32)
            make_identity(nc, ident)
            ident_r = persist.tile([P, P], F32R)
            nc.vector.tensor_copy(ident_r[:], ident[:])
            cos_sb = persist.tile([P, TT, D // 2], F32)   # [t%128, t//128, i]
            sin_sb = persist.tile([P, TT, D // 2], F32)
            nc.gpsimd.dma_start(cos_sb[:], cos_d.rearrange("(n p) i -> p n i", p=P))
            nc.gpsimd.dma_start(sin_sb[:], sin_d.rearrange("(n p) i -> p n i", p=P))
            onezero = persist.tile([P, 2], F32)
            nc.vector.memset(onezero[:, 0:1], 1.0)
            nc.vector.memset(onezero[:, 1:2], 0.0)

            e_const = persist.tile([P, 512], F32R)
            ec_f = persist.tile([P, 512], F32)
            nc.vector.memset(ec_f[:], 0.25)
            nc.vector.tensor_copy(e_const[:], ec_f[:])
            v_sb = persist.tile([P, TT, HL, D + 2], F32R)    # [k%128, k//128, h, d|1]
            qT = persist.tile([P, HL // 2, T], F32R)         # [dim in pair, pair, t]
            kT = persist.tile([P, HL // 2, T], F32R)

            # ---------------- phase 1: qkv projection + rope + transposes ----
            with (
                tc.tile_pool(name="wpool", bufs=1) as wpool,
                tc.tile_pool(name="xt", bufs=5) as xtpool,
                tc.tile_pool(name="qkvp", bufs=5, space="PSUM") as qkvp,
                tc.tile_pool(name="rope_tmp", bufs=3) as rtmp,
                tc.tile_pool(name="rot", bufs=3) as rotpool,
                tc.tile_pool(name="trp", bufs=3, space="PSUM") as trp,
            ):
                xt_r = xt_d.rearrange("(ko p) t -> p ko t", p=P)
                xt_tiles = {}

                def load_xt(tt):
                    xt_t = xtpool.tile(
                        [P, C // P, P], F32R, name=f"xt_{tt}", tag="xt"
                    )
                    nc.sync.dma_start(xt_t[:], xt_r[:, :, tt * P : (tt + 1) * P])
                    xt_tiles[tt] = xt_t

                GA = 4  # startup group: ko-outer over first GA tiles
                for tt in range(GA):
                    load_xt(tt)
                w_sb = wpool.tile([P, C // P, 3 * CL], F32R)  # 48KB/part
                wqkv_r = wqkv_d.rearrange("(ko p) n -> p ko n", p=P)
                # chunk-major arrival order: the startup group's Q matmuls
                # only need chunk 0, so land it before K/V weight columns
                for ch in range(3):
                    for ko in range(C // P):
                        nc.scalar.dma_start(
                            w_sb[:, ko, ch * 512 : (ch + 1) * 512],
                            wqkv_r[:, ko, ch * 512 : (ch + 1) * 512],
                        )
                if use_bias:
                    b_bc = wpool.tile([P, 3 * CL], F32R)
                    nc.sync.dma_start(
                        b_bc[:], bqkv_d[None, :].to_broadcast((P, 3 * CL))
                    )

                def postproc(tt, ch, ps):
                    if use_bias:
                        nc.vector.tensor_add(
                            ps[:], ps[:], b_bc[:, ch * 512 : (ch + 1) * 512]
                        )
                    if ch < 2:  # Q or K: rope on DVE straight from PSUM
                        eng = nc.vector
                        pv = ps.rearrange(
                            "p (h i two) -> p h i two", h=HL, two=2
                        )
                        e_, o_ = pv[:, :, :, 0], pv[:, :, :, 1]
                        cosb = cos_sb[:, tt, None, :].to_broadcast((P, HL, D // 2))
                        sinb = sin_sb[:, tt, None, :].to_broadcast((P, HL, D // 2))
                        rot = rotpool.tile([P, HL, D], F32R)
                        re_, ro_ = rot[:, :, 0 : D // 2], rot[:, :, D // 2 : D]
                        t1 = rtmp.tile([P, HL, D // 2], F32, tag="t1")
                        t2 = rtmp.tile([P, HL, D // 2], F32, tag="t2")
                        t3 = rtmp.tile([P, HL, D // 2], F32, tag="t3")
                        t4 = rtmp.tile([P, HL, D // 2], F32, tag="t4")
                        eng.tensor_mul(t1[:], e_, cosb)
                        eng.tensor_mul(t2[:], o_, sinb)
                        eng.tensor_mul(t3[:], e_, sinb)
                        eng.tensor_mul(t4[:], o_, cosb)
                        eng.tensor_sub(re_, t1[:], t2[:])
                        eng.tensor_add(ro_, t3[:], t4[:])
                        dstT = qT if ch == 0 else kT
                        rflat = rot.rearrange("p h d -> p (h d)")
                        tp2 = trp.tile([P, 512], F32R)
                        for pr in range(HL // 2):
                            nc.tensor.matmul(
                                tp2[:, pr * P : (pr + 1) * P],
                                rflat[:, pr * P : (pr + 1) * P],
                                ident_r[:],
                                is_transpose=True,
                                start=(pr == 0),
                                stop=(pr == 3),
                            )
                        nc.scalar.copy(
                            dstT[:, :, tt * P : (tt + 1) * P],
                            tp2.rearrange("p (b q) -> p b q", b=4),
                        )
                    else:  # V: copy into [t, h, d] layout + ones column
                        nc.scalar.copy(
                            v_sb[:, tt, :, 0:D],
                            ps.rearrange("p (h d) -> p h d", h=HL),
                        )
                        nc.vector.tensor_copy(
                            v_sb[:, tt, :, D : D + 2],
                            onezero[:, None, :].to_broadcast((P, HL, 2)),
                        )

                # startup group: ko-outer so each arriving 256KB weight block
                # feeds GA matmuls — PE overlaps the initial weight DMA
                # stream instead of stalling on the full 6MB load.
                for ch in range(3):
                    ga_ps = [
                        qkvp.tile([P, 512], F32, name=f"ga{ch}_{t}", tag="qkv")
                        for t in range(GA)
                    ]
                    for kb in range(C // P):
                        for t in range(GA):
                            nc.tensor.matmul(
                                ga_ps[t][:],
                                xt_tiles[t][:, kb, :],
                                w_sb[:, kb, ch * 512 : (ch + 1) * 512],
                                start=(kb == 0),
                                stop=(kb == C // P - 1),
                            )
                    for t in range(GA):
                        postproc(t, ch, ga_ps[t])
                for t in range(GA):
                    xt_tiles.pop(t)
                for tt in range(GA, min(GA + 3, TT)):
                    load_xt(tt)
                for tt in range(GA, TT):
                    if tt + 3 < TT:
                        load_xt(tt + 3)
                    xt_t = xt_tiles.pop(tt)
                    for ch in range(3):
                        ps = qkvp.tile([P, 512], F32, tag="qkv")
                        for kb in range(C // P):
                            nc.tensor.matmul(
                                ps[:],
                                xt_t[:, kb, :],
                                w_sb[:, kb, ch * 512 : (ch + 1) * 512],
                                start=(kb == 0),
                                stop=(kb == C // P - 1),
                            )
                        postproc(tt, ch, ps)

            if phases < 2:
                # timing-only build: dump a token result so outputs bind
                with tc.tile_pool(name="p1o", bufs=1) as p1o:
                    tok = p1o.tile([P, 512], F32)
                    nc.vector.tensor_copy(tok[:], qT[:, 0, 0:512])
                    nc.sync.dma_start(out_d[0:P, 0:512], tok[:])
                return

            # ------- phase 2+3: attention (qc-outer) + interleaved projection --
            with tc.tile_pool(name="p2", bufs=1) as p2:
                yT = p2.tile([P, HL // 2, T], F32R)
                # additive master causal mask: mm[p, c] = 0 iff c >= p + 896
                # else -1e30. slice [:, 896-r : 896-r+W] masks [j < p + r].
                mm = p2.tile([P, 1920], F32R)
                nc.gpsimd.dma_start(mm[:], mask_d[:])
                wp_sb = p2.tile([P, CL // P, C], F32R)
                for s in range(CL // P):
                    nc.sync.dma_start(
                        wp_sb[:, s, :],
                        wproj_d.rearrange("(s p) n -> p s n", p=P)[:, s, :],
                    )
                with (
                    tc.tile_pool(name="sp", bufs=3, space="PSUM") as spool,
                    tc.tile_pool(name="yp", bufs=1, space="PSUM") as ypool,
                    tc.tile_pool(name="ep", bufs=5) as epool,
                    tc.tile_pool(name="ys", bufs=3) as yspool,
                    tc.tile_pool(name="rp", bufs=3) as rpool,
                    tc.tile_pool(name="op", bufs=2) as opool,
                ):
                    for c in range(4):  # q chunks of 512
                        qs = c * 512
                        nk = 4 * (c + 1)   # k tiles covering q < qs+512
                        for hp in range(4):  # head pairs, interleaved
                            pair = (2 * hp, 2 * hp + 1)
                            y_ps = {}
                            qThs = {}
                            for x, hx in enumerate(pair):
                                tg = "AB"[x]
                                y_ps[hx] = ypool.tile(
                                    [D + 2, 512], F32,
                                    tag=f"y{tg}", name=f"y_ps{tg}"
                                )
                                po = (hx % 2) * D
                                pr = hx // 2
                                qThs[hx] = qT[po : po + D, pr, qs : qs + 512]
                            for kt in range(nk):
                                r = kt * P - qs
                                # fully-masked leading cols skipped; width
                                # stays >= 256 for full-rate fp32r
                                s_off = min(max(r, 0), 256)
                                re = r - s_off  # 0/128 residual vs mask tile
                                for x, hx in enumerate(pair):
                                    tg = "AB"[x]
                                    po = (hx % 2) * D
                                    pr = hx // 2
                                    kslice = kT[
                                        po : po + D, pr, kt * P : (kt + 1) * P
                                    ]
                                    vsl = v_sb[:, kt, hx, :]
                                    sp = spool.tile(
                                        [P, 512], F32,
                                        name=f"s{tg}", tag=f"s{tg}"
                                    )
                                    nc.tensor.matmul(
                                        sp[:, s_off:512], kslice,
                                        qThs[hx][:, s_off:512],
                                        start=True, stop=(r < 0),
                                    )
                                    if r >= 0:
                                        # additive causal mask in the same
                                        # accumulation group
                                        nc.tensor.matmul(
                                            sp[:, s_off:512],
                                            ident_r[:],
                                            mm[:, 896 - re : 1408 - re - s_off],
                                            start=False, stop=True,
                                        )
                                    e = epool.tile(
                                        [P, 512], F32R,
                                        name=f"e{tg}", tag=f"e{tg}"
                                    )
                                    if EXPERIMENT != 1:
                                        nc.scalar.activation(
                                            e[:, s_off:512], sp[:, s_off:512],
                                            mybir.ActivationFunctionType.Exp,
                                            scale=1.0 / math.sqrt(D),
                                        )
                                    rhs_e = e_const if EXPERIMENT == 1 else e
                                    nc.tensor.matmul(
                                        y_ps[hx][:, s_off:512], vsl,
                                        rhs_e[:, s_off:512],
                                        start=(kt == 0), stop=(kt == nk - 1),
                                    )
                            for hx in pair:
                                # drain PSUM fast (ACT copy) so the next
                                # pair reuses the bank; normalize off the
                                # critical path from SBUF.
                                po = (hx % 2) * D
                                pr = hx // 2
                                y_sb = yspool.tile(
                                    [D + 2, 512], F32, name="y_sb", tag="ysb"
                                )
                                nc.scalar.copy(y_sb[:], y_ps[hx][:])
                                recip = rpool.tile([1, 512], F32, tag="recip")
                                nc.vector.reciprocal(
                                    recip[:], y_sb[D : D + 1, :]
                                )
                                rb = rpool.tile([D, 512], F32, tag="rb")
                                nc.gpsimd.partition_broadcast(rb[:], recip[:])
                                nc.vector.tensor_mul(
                                    yT[po : po + D, pr, qs : qs + 512],
                                    y_sb[0:D, :],
                                    rb[:],
                                )
                        if phases < 3:
                            if c == 3:
                                o_sb = opool.tile([P, 512], F32)
                                nc.vector.tensor_copy(o_sb[:], yT[:, 0, 0:512])
                                nc.sync.dma_start(out_d[0:P, 0:512], o_sb[:])
                            continue
                        # projection for the t range this q chunk covers
                        for tt in range(c * 4, c * 4 + 4):
                            for nch in range(2):
                                # shares the sA PSUM ring (same shape/tag)
                                pps = spool.tile(
                                    [P, 512], F32, name="sA", tag="sA"
                                )
                                for sb in range(CL // P):
                                    nc.tensor.matmul(
                                        pps[:],
                                        yT[:, sb, tt * P : (tt + 1) * P],
                                        wp_sb[:, sb, nch * 512 : (nch + 1) * 512],
                                        start=(sb == 0),
                                        stop=(sb == CL // P - 1),
                                    )
                                o_sb = opool.tile([P, 512], F32)
                                nc.vector.tensor_copy(o_sb[:], pps[:])
                                nc.sync.dma_start(
                                    out_d[
                                        tt * P : (tt + 1) * P,
                                        nch * 512 : (nch + 1) * 512,
                                    ],
                                    o_sb[:],
                                )


def _round_f32r(a):
    """Round fp32 to the fp32r storage format: RNE to 11 mantissa bits
    (low 12 mantissa bits zero)."""
    u = np.ascontiguousarray(a, dtype=np.float32).view(np.uint32)
    r = u + (0x7FF + ((u >> 12) & 1))
    r &= np.uint32(0xFFFFF000)
    return r.view(np.float32)


def _host_consts():
    i = np.arange(D // 2, dtype=np.float32)
    theta = (10000.0 ** (-2.0 * i / D)).astype(np.float32)
    pos = np.arange(T, dtype=np.float32)[:, None]
    ang = pos * theta[None, :]
    cos_t = np.cos(ang).astype(np.float32)
    sin_t = np.sin(ang).astype(np.float32)
    p = np.arange(P)[:, None]
    c = np.arange(1920)[None, :]
    # additive master causal mask: 0 where valid, -1e30 where masked
    masks = np.where(c >= p + 896, 0.0, -1e30).astype(np.float32)
    return cos_t, sin_t, masks


def _make_in_maps(x, W_qkv, b_qkv, W_proj):
    cos_t, sin_t, masks = _host_consts()
    in_maps = []
    for core in range(N_CORES):
        b = core // 2
        g = core % 2
        cols = np.arange(g * CL, (g + 1) * CL)  # local head cols within Q/K/V
        wq = W_qkv[:, cols]
        wk = W_qkv[:, C + cols]
        wv = W_qkv[:, 2 * C + cols]
        wqkv = _round_f32r(np.concatenate([wq, wk, wv], axis=1))
        bq = _round_f32r(
            np.concatenate([b_qkv[cols], b_qkv[C + cols], b_qkv[2 * C + cols]])
        )
        wproj = _round_f32r(W_proj[cols, :])
        in_maps.append(
            {
                "xt_in": _round_f32r(x[b].T),
                "wqkv": wqkv,
                "bqkv": np.ascontiguousarray(bq),
                "wproj": wproj,
                "cos_t": cos_t,
                "sin_t": sin_t,
                "masks": masks,
            }
        )
    return in_maps


class _Runner:
    """Cached jitted shard_map executable over the 8 NeuronCores."""

    def __init__(self, nc, n_cores):
        import jax
        from jax.sharding import Mesh, PartitionSpec, NamedSharding
        from jax.experimental.shard_map import shard_map
        from concourse import bass2jax

        bass2jax.install_neuronx_cc_hook()
        self.n_cores = n_cores
        partition_name = (
            nc.partition_id_tensor.name if nc.partition_id_tensor else None
        )
        in_names, out_names, out_avals, zeros = [], [], [], []
        for alloc in nc.m.functions[0].allocations:
            if not isinstance(alloc, mybir.MemoryLocationSet):
                continue
            name = alloc.memorylocations[0].name
            if alloc.kind == "ExternalInput":
                if name != partition_name:
                    in_names.append(name)
            elif alloc.kind == "ExternalOutput":
                shape = tuple(alloc.tensor_shape)
                dtype = mybir.dt.np(alloc.dtype)
                out_names.append(name)
                out_avals.append(jax.core.ShapedArray(shape, dtype))
                zeros.append(np.zeros(shape, dtype))
        self.in_names, self.out_names, self.out_avals = (
            in_names,
            out_names,
            out_avals,
        )
        all_in = list(in_names) + list(out_names)
        if partition_name is not None:
            all_in.append(partition_name)

        def _body(*args):
            operands = list(args)
            if partition_name is not None:
                operands.append(bass2jax.partition_id_tensor())
            return tuple(
                bass2jax._bass_exec_p.bind(
                    *operands,
                    out_avals=tuple(out_avals),
                    in_names=tuple(all_in),
                    out_names=tuple(out_names),
                    lowering_input_output_aliases=(),
                    sim_require_finite=True,
                    sim_require_nnan=True,
                    nc=nc,
                )
            )

        devices = jax.devices()[:n_cores]
        mesh = Mesh(np.asarray(devices), ("core",))
        spec = PartitionSpec("core")
        self.sharding = NamedSharding(mesh, spec)
        n_outs = len(out_names)
        self.fn = jax.jit(
            shard_map(
                _body,
                mesh=mesh,
                in_specs=(spec,) * (len(in_names) + n_outs),
                out_specs=(spec,) * n_outs,
                check_rep=False,
            ),
            keep_unused=True,
        )
        self._jax = jax
        self.zeros_dev = [
            jax.device_put(
                np.zeros((n_cores * z.shape[0], *z.shape[1:]), z.dtype),
                self.sharding,
            )
            for z in zeros
        ]

    def run(self, in_maps):
        jax = self._jax
        concat = [
            np.concatenate([np.asarray(m[name]) for m in in_maps], axis=0)
            for name in self.in_names
        ]
        dev_in = [jax.device_put(a, self.sharding) for a in concat]
        outs = self.fn(*dev_in, *self.zeros_dev)
        results = []
        for c in range(self.n_cores):
            d = {}
            for i, name in enumerate(self.out_names):
                shape = self.out_avals[i].shape
                d[name] = np.asarray(outs[i]).reshape(self.n_cores, *shape)[c]
            results.append(d)
        return results


def kernel(x, W_qkv, b_qkv, W_proj, b_proj):
    x = np.asarray(x, dtype=np.float32)
    W_qkv = np.asarray(W_qkv, dtype=np.float32)
    b_qkv = np.asarray(b_qkv, dtype=np.float32)
    W_proj = np.asarray(W_proj, dtype=np.float32)
    b_proj = np.asarray(b_proj, dtype=np.float32)

    use_bias = bool(np.any(b_qkv))
    key = ("runner", use_bias)
    if key not in _CACHE:
        _CACHE[("nc", use_bias)] = build_nc(use_bias)
        _CACHE[key] = _Runner(_CACHE[("nc", use_bias)], N_CORES)
    runner = _CACHE[key]

    in_maps = _make_in_maps(x, W_qkv, b_qkv, W_proj)
    results = runner.run(in_maps)
    out = np.empty((B, T, C), dtype=np.float32)
    for b in range(B):
        out[b] = results[2 * b]["out_p"] + results[2 * b + 1]["out_p"]
    out += b_proj[None, None, :]
    return out

